# revision 2
# baseline (speedup 1.0000x reference)
"""Multi-head attention (b=2, n=2048, d_model=1024, h=16, d_k=d_v=64) + relu(fc) +
residual + LayerNorm, sharded over 8 NeuronCores.

Sharding: core i = (batch bi = i//4) x (head-group hg = i%4, 4 heads each).

v2 changes vs baseline:
- input DMAs split per 512-seq slab and ordered weights-first so the first
  projection matmul starts ~2us in instead of ~50us.
- score matmuls alternate PE row groups (heads 2p / 2p+1) every instruction so
  LDWEIGHTS overlaps the previous matmul (measured 216ns vs 336ns per MM).
- context matmuls run in fp8 DoubleRow mode: 256-key contraction per MM
  (pse + vh quantized to e4m3; ones column folds the softmax denominator in).
- fc matmuls run in fp8 DoubleRow (256-dim contraction per MM); w_fc is
  pre-scaled x64 on the host to dodge e4m3 subnormals, descaled at psum copy.
- software pipelining: per group g the PE stream is scores(g), ctx(g-1), with
  v_proj / qk_proj(1) / fc matmuls as fillers, because exp on the scalar
  engine (18.4us/tile) outpaces the attention matmuls (12.5us/tile).

v3 changes:
- q/k/v inputs and w_qs/w_ks/w_vs arrive fp8 e4m3 (weights pre-scaled x64/x32
  on host to dodge subnormals; all scales are powers of two so they fold
  exactly into the exp scale and the fc descale).  Projection matmuls run in
  fp8 DoubleRow (256-contraction per MM): halves proj PE time and input DMA.
- numpy-simulated end-to-end error for the full fp8 pipeline: 1.2e-3 vs the
  2e-2 gate (softmax cancels common-mode exp/quantization error).
"""

import numpy as np
import ml_dtypes
from contextlib import ExitStack

B = 2
N = 2048
D = 1024
H = 16
DK = 64
HL = H // 4          # heads per core
CSL = HL * DK        # 256 per-core fc contraction
ROWS = N // 4        # 512 output rows per core
LN_EPS = 1e-6
N_CORES = 8
WFC_SCALE = 64.0     # host pre-scale on w_fc (fp8 subnormal dodge)
WQK_SCALE = 64.0     # host pre-scale on w_qs/w_ks -> scores come out x4096
WV_SCALE = 32.0      # host pre-scale on w_vs -> vh/ctx come out x32

_CACHE = {}


def _build():
    import concourse.bass as bass
    import concourse.tile as tile
    import concourse.mybir as mybir
    from concourse import bacc

    bf16 = mybir.dt.bfloat16
    f32 = mybir.dt.float32
    fp8 = mybir.dt.float8e4
    AF = mybir.ActivationFunctionType
    Alu = mybir.AluOpType
    DR = mybir.MatmulPerfMode.DoubleRow

    nc = bacc.Bacc("TRN2", target_bir_lowering=False, debug=False,
                   num_devices=N_CORES)

    KCl = D // 128
    # weights/residual arrive pre-arranged from the host in SBUF layout so
    # their DMAs are contiguous 2-4KB-per-partition line-rate transfers
    qT = nc.dram_tensor("qT", [D, N], fp8, kind="ExternalInput").ap()
    kT = nc.dram_tensor("kT", [D, N], fp8, kind="ExternalInput").ap()
    vT = nc.dram_tensor("vT", [D, N], fp8, kind="ExternalInput").ap()
    wq = nc.dram_tensor("wq", [128, KCl, CSL], fp8, kind="ExternalInput").ap()
    wk = nc.dram_tensor("wk", [128, KCl, CSL], fp8, kind="ExternalInput").ap()
    wv = nc.dram_tensor("wv", [128, KCl, CSL], fp8, kind="ExternalInput").ap()
    wfc8 = nc.dram_tensor("wfc8", [128, CSL // 128, D], fp8, kind="ExternalInput").ap()
    qres = nc.dram_tensor("qres", [128, N // 512, D], f32, kind="ExternalInput").ap()
    gamma = nc.dram_tensor("gamma", [D], f32, kind="ExternalInput").ap()
    beta = nc.dram_tensor("beta", [D], f32, kind="ExternalInput").ap()
    y = nc.dram_tensor("y", [ROWS, D], f32, kind="ExternalOutput").ap()

    KC = D // 128     # 8 contraction chunks for projections
    KCP = KC // 2     # 4 DoubleRow chunk-pairs for projections
    ST = N // 512     # 4 seq tiles of 512 queries
    SC = N // 128     # 16 seq chunks of 128 keys
    G = 2             # key chunks per exp batch / DR pair
    NG = SC // G      # 8 groups per attention tile
    # raw scores in PSUM are x(WQK_SCALE^2); fold into the exp scale exactly
    EXP_SCALE = 1.0 / (float(np.sqrt(DK)) * WQK_SCALE * WQK_SCALE)
    # Schraudolph exp straight into e4m3 bits: u8 = round(A8*x + B8) as int8,
    # bitcast to fp8.  One DVE tensor_scalar per slice; numpy sim shows the
    # softmax cancels the approx error (rel 1.1e-3, same as exact exp).
    EXP8_A = 8.0 * float(np.log2(np.e)) * EXP_SCALE
    EXP8_B = 8.0 * (7.0 - 0.45)

    def exp_on_dve(g, s):
        # 4 of 16 slices per attention tile go to the DVE, spread out so
        # neither engine gets a long exclusive run
        return g % 2 == 1 and s == (g // 2) % 2

    with tile.TileContext(nc) as tc:
        with ExitStack() as ctx:
            persist = ctx.enter_context(tc.tile_pool(name="persist", bufs=1))
            work = ctx.enter_context(tc.tile_pool(name="work", bufs=2))
            epool = ctx.enter_context(tc.tile_pool(name="epool", bufs=4))
            pat = ctx.enter_context(tc.tile_pool(name="pat", bufs=1, space="PSUM"))
            dram = ctx.enter_context(tc.tile_pool(name="dram", bufs=2, space="DRAM"))
            late_ctx = ExitStack()
            late = late_ctx.enter_context(tc.tile_pool(name="late", bufs=1))
            qkv_ctx = ExitStack()
            qkv = qkv_ctx.enter_context(tc.tile_pool(name="qkv", bufs=1))

            # PSUM: "s" [128,2,512] x3 = 6 banks (scores; proj/fc borrow these
            # slots), "c" [65,512] x2 = 2 banks (ctx accumulators).
            def ps_s():
                return pat.tile([128, G, 512], f32, tag="s", name="ps_s", bufs=3)

            def ps_c():
                # full 128 partitions: ctx uses [0:DK+1], the HAM keepalive
                # matmuls write the otherwise-unused [96:128] strip
                return pat.tile([128, 512], f32, tag="c", name="ps_c", bufs=2)

            def ps_f(n=512):
                return pat.tile([128, n], f32, tag="s", name="ps_f", bufs=3)

            # ---- input loads: weights first, then per-slab slices ------------
            wq_sb = qkv.tile([128, KC, CSL], fp8, tag="wq", name="wq")
            wk_sb = qkv.tile([128, KC, CSL], fp8, tag="wk", name="wk")
            wv_sb = qkv.tile([128, KC, CSL], fp8, tag="wv", name="wv")
            nc.sync.dma_start(out=wq_sb, in_=wq)
            nc.sync.dma_start(out=wk_sb, in_=wk)

            qT_sb = qkv.tile([128, KC, N], fp8, tag="qT", name="qT")
            kT_sb = qkv.tile([128, KC, N], fp8, tag="kT", name="kT")
            vT_sb = qkv.tile([128, KC, N], fp8, tag="vT", name="vT")
            # halves interleaved: q/k half 0 first (first proj + attention
            # groups 0-3), then v half 0 (vhp for those groups), then the
            # second halves
            # one 2MB DMA per tensor (transfers under ~1MiB are descriptor-
            # dominated): out[p, kc, n] = in[kc*128 + p, n]
            def xT_ap(x):
                return bass.AP(tensor=x.tensor, offset=x.offset,
                               ap=[[N, 128], [128 * N, KC], [1, N]])

            nc.sync.dma_start(out=qT_sb, in_=xT_ap(qT))
            nc.sync.dma_start(out=kT_sb, in_=xT_ap(kT))
            nc.sync.dma_start(out=wv_sb, in_=wv)
            nc.sync.dma_start(out=vT_sb, in_=xT_ap(vT))

            # late inputs on the scalar HWDGE ring so they don't delay the above
            wfc_sb = late.tile([128, CSL // 128, D], fp8, tag="wfc", name="wfc")
            nc.scalar.dma_start(out=wfc_sb, in_=wfc8)
            qres_sb = late.tile([128, ST, D], f32, tag="qres", name="qres")
            nc.scalar.dma_start(out=qres_sb, in_=qres)
            gamma_sb = late.tile([128, D], f32, tag="gamma", name="gamma")
            nc.scalar.dma_start(out=gamma_sb,
                                in_=bass.AP(tensor=gamma.tensor, offset=gamma.offset,
                                            ap=[[0, 128]] + gamma.ap))
            beta_sb = late.tile([128, D], f32, tag="beta", name="beta")
            nc.scalar.dma_start(out=beta_sb,
                                in_=bass.AP(tensor=beta.tensor, offset=beta.offset,
                                            ap=[[0, 128]] + beta.ap))
            xacc = qres_sb  # relu+residual accumulates in place over the residual

            # HAM warm-up: dependency-free matmuls on memset data span the
            # input-DMA wait so the PE clock is at 2.4GHz (not the cold
            # 1.2GHz) when the first projection matmuls arrive, and the
            # 4096-cycle activity window never lapses during the DMA-paced
            # startup phase.
            warm = qkv.tile([128, 512], bf16, tag="warm", name="warm")
            nc.vector.memset(warm, 0.0)
            for _ in range(48):
                nc.tensor.matmul(ps_f()[:, 0:256], warm[:, 0:128],
                                 warm[:, 0:256], start=True, stop=True)

            # ---- projections -------------------------------------------------
            # qhT/khT: [dk, seq] per head, heads 2p / 2p+1 stacked on partition
            # halves.  k copies go through the scalar engine pre-attention so
            # DVE and ACT split the psum-evacuation work.
            qhT = [persist.tile([128, N], bf16, tag=f"qhT{p}", name=f"qhT{p}") for p in range(2)]
            khT = [persist.tile([128, N], bf16, tag=f"khT{p}", name=f"khT{p}") for p in range(2)]

            def qk_proj_pair(p, st, use_act=False):
                # q and k chains interleaved on two psum banks: consecutive
                # MMs never hit the same bank (WAW chain penalty) and LDW
                # overlaps the previous matmul.  fp8 DoubleRow: 256 of the
                # 1024 contraction dims per MM (chunk-pairs 2kcp/2kcp+1).
                sl = slice(st * 512, (st + 1) * 512)
                psq = ps_f()
                psk = ps_f()
                for kcp in range(KCP):
                    cp = slice(2 * kcp, 2 * kcp + 2)
                    nc.tensor.matmul(
                        psq, wq_sb[:, cp, p * 128:(p + 1) * 128],
                        qT_sb[:, cp, sl],
                        start=(kcp == 0), stop=(kcp == KCP - 1),
                        perf_mode=DR)
                    nc.tensor.matmul(
                        psk, wk_sb[:, cp, p * 128:(p + 1) * 128],
                        kT_sb[:, cp, sl],
                        start=(kcp == 0), stop=(kcp == KCP - 1),
                        perf_mode=DR)
                nc.vector.tensor_copy(out=qhT[p][:, sl], in_=psq)
                if use_act:
                    nc.scalar.copy(out=khT[p][:, sl], in_=psk)
                else:
                    nc.vector.tensor_copy(out=khT[p][:, sl], in_=psk)

            # vh pairs for DoubleRow ctx: [128 keys, 2 chunks, HL heads, 80]
            # (65 used: 64 dims + ones column; 80 keeps the DR pair stride
            # 16B-aligned).
            vhp = [persist.tile([128, G, HL, 80], fp8, tag=f"vhp{g}", name=f"vhp{g}")
                   for g in range(NG)]
            for g in range(NG):
                nc.vector.memset(vhp[g][:, :, :, DK:DK + 1], 1.0)

            def v_proj_pair(g):
                # chunks 2g / 2g+1 interleaved on two psum banks (fp8 DR)
                sc0 = 2 * g
                psa = ps_f(CSL)
                psb = ps_f(CSL)
                for kcp in range(KCP):
                    cp = slice(2 * kcp, 2 * kcp + 2)
                    nc.tensor.matmul(
                        psa, vT_sb[:, cp, sc0 * 128:(sc0 + 1) * 128],
                        wv_sb[:, cp, :],
                        start=(kcp == 0), stop=(kcp == KCP - 1),
                        perf_mode=DR)
                    nc.tensor.matmul(
                        psb, vT_sb[:, cp, (sc0 + 1) * 128:(sc0 + 2) * 128],
                        wv_sb[:, cp, :],
                        start=(kcp == 0), stop=(kcp == KCP - 1),
                        perf_mode=DR)
                nc.vector.tensor_copy(
                    out=vhp[g][:, 0, :, 0:DK],
                    in_=psa.rearrange("p (h d) -> p h d", h=HL))
                nc.vector.tensor_copy(
                    out=vhp[g][:, 1, :, 0:DK],
                    in_=psb.rearrange("p (h d) -> p h d", h=HL))

            # ctxn: normalized context, fp8, [128 c, 2 cc-chunks, seq] --
            # cc chunk index == p (heads 2p, 2p+1 on partition halves).
            ctxn = persist.tile([128, 2, N], fp8, tag="ctxn", name="ctxn")
            ones_sb = persist.tile([1, DK], bf16, tag="ones", name="ones")
            nc.vector.memset(ones_sb, 1.0)

            def attention(p, t, extra=None):
                pc = [ps_c() for _ in range(2)]
                pse_prev = [None, None]

                def ctx_mm(g, s):
                    nc.tensor.matmul(
                        pc[s][0:DK + 1, :],
                        vhp[g][:, :, 2 * p + s, 0:DK + 1],
                        pse_prev[s],
                        start=(g == 0), stop=(g == NG - 1),
                        perf_mode=DR)

                for g in range(NG):
                    # fillers first: they are dependency-free, so they run
                    # while the scores MMs behind them wait on psum drain --
                    # keeping the PE busy (HAM stays at full clock)
                    if extra is not None:
                        extra(g)
                    if g >= 1:
                        # HAM keepalive: a tiny dependency-free matmul into
                        # the unused partitions 96-127 of the ctx accumulator
                        # bank.  It absorbs the PE stall at the psum-slot
                        # rotation so the activity monitor never re-throttles
                        # the clock to 1.2GHz mid-kernel.
                        nc.tensor.matmul(
                            pc[g % 2][96:128, 0:256], warm[:, 0:32],
                            warm[:, 0:256], start=True, stop=True,
                            tile_position=(0, 96), skip_group_check=True)
                    ppss = [ps_s(), ps_s()]
                    # scores: alternate row groups every MM so LDW overlaps
                    for j in range(G):
                        kc = g * G + j
                        for s in range(2):
                            lo = 64 * s
                            nc.tensor.matmul(
                                ppss[s][:, j, :],
                                khT[p][lo:lo + 64, kc * 128:(kc + 1) * 128],
                                qhT[p][lo:lo + 64, t * 512:(t + 1) * 512],
                                start=True, stop=True,
                                tile_position=(lo, 0))
                    # ctx for the previous group (exp ran during these scores)
                    if g > 0:
                        for s in range(2):
                            ctx_mm(g - 1, s)
                    for s in range(2):
                        if exp_on_dve(g, s):
                            u8 = epool.tile([128, G, 512], mybir.dt.int8,
                                            tag="e8", name="e8")
                            nc.vector.tensor_scalar(
                                out=u8, in0=ppss[s], scalar1=EXP8_A,
                                scalar2=EXP8_B, op0=Alu.mult, op1=Alu.add)
                            pse_prev[s] = u8.bitcast(fp8)
                        else:
                            pse = epool.tile([128, G, 512], fp8, tag="e", name="e")
                            nc.scalar.activation(out=pse, in_=ppss[s], func=AF.Exp,
                                                 scale=EXP_SCALE)
                            pse_prev[s] = pse
                for s in range(2):
                    ctx_mm(NG - 1, s)

                # normalize: broadcast the ones-row (softmax denominator)
                # across 64 partitions with a K=1 ones-matmul on the PE, take
                # the fast approx reciprocal (1.2 cpe vs 6), multiply into the
                # fp8 ctxn tile.  (A stride-0 DRAM-bounce broadcast would free
                # the PE matmul, but the hazard tracker misses the ordering on
                # the stride-0 read -> races -> NaN.  Keep the PE path.)
                for s in range(2):
                    d1 = work.tile([1, 512], bf16, tag="rb", name="rb")
                    nc.scalar.copy(out=d1, in_=pc[s][DK:DK + 1, :])
                    ps_rb = ps_f()
                    nc.tensor.matmul(ps_rb[0:DK, :], ones_sb, d1,
                                     start=True, stop=True)
                    rb = work.tile([DK, 512], f32, tag="rb64", name="rb64")
                    nc.vector.reciprocal_approx_fast(out=rb, in_=ps_rb[0:DK, :])
                    nc.vector.tensor_mul(
                        out=ctxn[64 * s:64 * (s + 1), p, t * 512:(t + 1) * 512],
                        in0=pc[s][0:DK, :], in1=rb)

            # fc (fp8 DoubleRow, both 128-chunks contracted per MM) + chunked
            # ReduceScatter per slab; each core ends with rows
            # [t*512 + rank*128, +128) of its batch.
            def fc_mm(t, qq, nh):
                qc = t * 4 + qq
                ps = ps_f()
                nc.tensor.matmul(
                    ps,
                    ctxn[:, :, qc * 128:(qc + 1) * 128],
                    wfc_sb[:, :, nh * 512:(nh + 1) * 512],
                    start=True, stop=True, perf_mode=DR)
                fcs = work.tile([128, 512], fp8, tag="fcs", name="fcs", bufs=8)
                # descale alternates engines to balance ACT vs DVE load
                if (qq + nh) % 2 == 0:
                    nc.scalar.activation(out=fcs, in_=ps, func=AF.Copy,
                                         scale=1.0 / (WFC_SCALE * WV_SCALE))
                else:
                    nc.vector.tensor_scalar(out=fcs, in0=ps,
                                            scalar1=1.0 / (WFC_SCALE * WV_SCALE),
                                            scalar2=None, op0=Alu.mult)
                return fcs

            rs_outs = []

            def fc_rs(t, fcs_parts):
                rs_in = dram.tile([512, D], fp8, tag="rs_in", name="rs_in",
                                  bufs=4)
                rs_out = dram.tile([128, D], fp8, tag="rs_out", name="rs_out",
                                   bufs=4)
                for (qq, nh), fcs in fcs_parts.items():
                    nc.sync.dma_start(
                        out=rs_in[qq * 128:(qq + 1) * 128, nh * 512:(nh + 1) * 512],
                        in_=fcs)
                nc.gpsimd.collective_compute(
                    "ReduceScatter",
                    mybir.AluOpType.add,
                    replica_groups=[[0, 1, 2, 3], [4, 5, 6, 7]],
                    ins=[rs_in.opt()],
                    outs=[rs_out.opt()])
                rs_outs.append(rs_out)

            # ---- relu/residual + layernorm for one 128-row output slab ------
            # rstd on the DVE (bit-trick seed + 2 Newton iterations) keeps the
            # scalar engine's ACT table on the exp set for the whole kernel.
            i32 = mybir.dt.int32
            MAGIC = float(0x5F3759DF)

            def ln_slab(t, gate):
                rs_sb = work.tile([128, D], fp8, tag="rs_sb", name="rs_sb",
                                  bufs=2)
                # corner-write gate: a data dep on attention progress keeps
                # this collective-waiting load (and the DVE ops behind it)
                # from hoisting into earlier engine-FIFO slots.
                nc.gpsimd.tensor_copy(out=rs_sb[0:1, 0:1], in_=gate)
                nc.gpsimd.dma_start(out=rs_sb, in_=rs_outs[t])
                nc.vector.scalar_tensor_tensor(
                    out=xacc[:, t, :], in0=rs_sb, scalar=0.0,
                    in1=qres_sb[:, t, :], op0=Alu.max, op1=Alu.add)
                x = xacc[:, t, :]
                stats = work.tile([128, 2, 6], f32, tag="stats", name="stats")
                nc.vector.bn_stats(out=stats[:, 0, :], in_=x[:, 0:512])
                nc.vector.bn_stats(out=stats[:, 1, :], in_=x[:, 512:1024])
                mv = work.tile([128, 2], f32, tag="mv", name="mv")
                nc.vector.bn_aggr(out=mv, in_=stats)
                v = work.tile([128, 1], f32, tag="veps", name="veps")
                nc.vector.tensor_scalar(out=v, in0=mv[:, 1:2], scalar1=LN_EPS,
                                        scalar2=None, op0=Alu.add)
                # rsqrt: y0 = bitcast(0x5F3759DF - bits(v)/2) via f32 arithmetic
                # on the bit pattern (exact enough for a seed), then 2x Newton.
                si = work.tile([128, 1], i32, tag="rss", name="rss")
                nc.vector.tensor_scalar(out=si, in0=v.bitcast(i32), scalar1=-0.5,
                                        scalar2=MAGIC, op0=Alu.mult, op1=Alu.add)
                yk = si.bitcast(f32)
                for _ in range(2):
                    a = work.tile([128, 1], f32, tag="rsa", name="rsa")
                    nc.vector.tensor_mul(out=a, in0=yk, in1=yk)
                    nc.vector.tensor_mul(out=a, in0=a, in1=v)
                    nc.vector.tensor_scalar(out=a, in0=a, scalar1=-0.5,
                                            scalar2=1.5, op0=Alu.mult, op1=Alu.add)
                    yn = work.tile([128, 1], f32, tag="rsy", name="rsy")
                    nc.vector.tensor_mul(out=yn, in0=yk, in1=a)
                    yk = yn
                xo = work.tile([128, D], f32, tag="xo", name="xo")
                nc.vector.tensor_scalar(out=xo, in0=x,
                                        scalar1=mv[:, 0:1], scalar2=yk,
                                        op0=Alu.subtract, op1=Alu.mult)
                nc.vector.tensor_mul(out=xo, in0=xo, in1=gamma_sb)
                nc.vector.tensor_add(out=xo, in0=xo, in1=beta_sb)
                nc.sync.dma_start(out=y[t * 128:(t + 1) * 128, :], in_=xo)

            # ---- schedule ----------------------------------------------------
            for st in range(ST):
                qk_proj_pair(0, st, use_act=True)
            v_proj_pair(0)
            v_proj_pair(1)

            # A(0,0) group g consumes vhp[g]; emit vhp[g+2] at group g, then
            # start qk_proj(1) in the last two group slots.
            def extra00(g):
                if g + 2 < NG:
                    v_proj_pair(g + 2)
                elif g == 6:
                    qk_proj_pair(1, 0)
                elif g == 7:
                    qk_proj_pair(1, 1)
            attention(0, 0, extra=extra00)

            def extra10(g):
                if g == 2:
                    qk_proj_pair(1, 2)
                elif g == 5:
                    qk_proj_pair(1, 3)
            attention(1, 0, extra=extra10)

            # fc for slab t-1: 4 parts as PE fillers in each of A(0,t) and
            # A(1,t) so both attention tiles keep the PE dense; ReduceScatter
            # fires after A(1,t).  LayerNorm for slab t-2 interleaves between
            # A(0,t) and A(1,t): its collective fired ~1.5 attention tiles
            # earlier, which covers cross-core skew, so the DVE ops behind it
            # don't head-of-line block.
            fcs_pending = {}

            def extra_fc(t_prev, half):
                def fill(g):
                    if g % 2 == 1:
                        part = 4 * half + g // 2
                        qq, nh = divmod(part, 2)
                        fcs_pending[(qq, nh)] = fc_mm(t_prev, qq, nh)
                return fill

            for t in range(1, ST):
                attention(0, t, extra=extra_fc(t - 1, 0))
                attention(1, t, extra=extra_fc(t - 1, 1))
                fc_rs(t - 1, fcs_pending)
                fcs_pending = {}
                if t >= 2:
                    ln_slab(t - 2, ctxn[0:1, 1, (t + 1) * 512 - 1:(t + 1) * 512])
            for part in range(8):
                qq, nh = divmod(part, 2)
                fcs_pending[(qq, nh)] = fc_mm(ST - 1, qq, nh)
            fc_rs(ST - 1, fcs_pending)
            qkv_ctx.close()

            # ---- tail: last two layernorm slabs ------------------------------
            last_gate = ctxn[0:1, 1, N - 1:N]
            ln_slab(2, last_gate)
            ln_slab(3, last_gate)
            late_ctx.close()

    nc.compile()
    return nc


def kernel(q, k, v, w_qs, w_ks, w_vs, w_fc, ln_gamma, ln_beta):
    from concourse import bass_utils

    if "nc" not in _CACHE:
        _CACHE["nc"] = _build()
    nc = _CACHE["nc"]

    bf = ml_dtypes.bfloat16
    f8 = ml_dtypes.float8_e4m3
    q = np.asarray(q, np.float32)
    k = np.asarray(k, np.float32)
    v = np.asarray(v, np.float32)
    w_fc = np.asarray(w_fc, np.float32)

    def to_f8(a):
        return np.clip(a, -448.0, 448.0).astype(f8)

    def warr(w, cs, scale):
        # [D, CSL] -> SBUF layout [128, KC, CSL], pre-scaled fp8
        a = np.asarray(w, np.float32)[:, cs].reshape(D // 128, 128, CSL) * scale
        return to_f8(np.ascontiguousarray(a.transpose(1, 0, 2)))

    in_maps = []
    for i in range(N_CORES):
        bi, hg = i // 4, i % 4
        cs = slice(hg * CSL, (hg + 1) * CSL)
        row_idx = np.concatenate(
            [np.arange(t * 512 + hg * 128, t * 512 + (hg + 1) * 128) for t in range(4)])
        wfc_a = (w_fc[cs, :] * WFC_SCALE).reshape(CSL // 128, 128, D)
        qres_a = q[bi][row_idx].reshape(4, 128, D)
        in_maps.append({
            "qT": to_f8(np.ascontiguousarray(q[bi].T)),
            "kT": to_f8(np.ascontiguousarray(k[bi].T)),
            "vT": to_f8(np.ascontiguousarray(v[bi].T)),
            "wq": warr(w_qs, cs, WQK_SCALE),
            "wk": warr(w_ks, cs, WQK_SCALE),
            "wv": warr(w_vs, cs, WV_SCALE),
            "wfc8": to_f8(np.ascontiguousarray(wfc_a.transpose(1, 0, 2))),
            "qres": np.ascontiguousarray(qres_a.transpose(1, 0, 2)).astype(np.float32),
            "gamma": np.ascontiguousarray(np.asarray(ln_gamma, np.float32)),
            "beta": np.ascontiguousarray(np.asarray(ln_beta, np.float32)),
        })

    run_kwargs = dict(_CACHE.get("run_kwargs", {}))
    res = bass_utils.run_bass_kernel_spmd(nc, in_maps, core_ids=list(range(N_CORES)),
                                          **run_kwargs)
    _CACHE["last_res"] = res
    out = np.empty((B, N, D), np.float32)
    for i in range(N_CORES):
        bi, hg = i // 4, i % 4
        yi = res.results[i]["y"]
        for t in range(4):
            out[bi, t * 512 + hg * 128:t * 512 + (hg + 1) * 128, :] = \
                yi[t * 128:(t + 1) * 128, :]
    return out



# revision 3
# speedup vs baseline: 1.0178x; 1.0178x over previous
"""Multi-head attention (b=2, n=2048, d_model=1024, h=16, d_k=d_v=64) + relu(fc) +
residual + LayerNorm, sharded over 8 NeuronCores.

Sharding: core i = (batch bi = i//4) x (head-group hg = i%4, 4 heads each).

v2 changes vs baseline:
- input DMAs split per 512-seq slab and ordered weights-first so the first
  projection matmul starts ~2us in instead of ~50us.
- score matmuls alternate PE row groups (heads 2p / 2p+1) every instruction so
  LDWEIGHTS overlaps the previous matmul (measured 216ns vs 336ns per MM).
- context matmuls run in fp8 DoubleRow mode: 256-key contraction per MM
  (pse + vh quantized to e4m3; ones column folds the softmax denominator in).
- fc matmuls run in fp8 DoubleRow (256-dim contraction per MM); w_fc is
  pre-scaled x64 on the host to dodge e4m3 subnormals, descaled at psum copy.
- software pipelining: per group g the PE stream is scores(g), ctx(g-1), with
  v_proj / qk_proj(1) / fc matmuls as fillers, because exp on the scalar
  engine (18.4us/tile) outpaces the attention matmuls (12.5us/tile).

v3 changes (302us -> ~260us):
- q/k/v inputs and w_qs/w_ks/w_vs arrive fp8 e4m3 (weights pre-scaled x64/x32
  on host to dodge subnormals; all scales are powers of two so they fold
  exactly into the exp scale and the fc descale).  Projection matmuls run in
  fp8 DoubleRow (256-contraction per MM): halves proj PE time and input DMA.
- 4/16 exp slices per attention tile run on the DVE as Schraudolph exp
  straight into e4m3 bits (one int8 tensor_scalar + bitcast); softmax cancels
  the approx error (measured rel err 1.7e-3 vs the 2e-2 gate).
- HAM keepalives: a 107ns dependency-free matmul into the unused partitions
  96-127 of the ctx psum bank before each scores group.  Without them the PE
  micro-stalls at the psum-slot rotation re-throttle the clock to 1.2GHz
  mid-kernel (HAM MID window) and the whole middle runs at half speed.
- layernorm for slab t-2 interleaves after A(1,t)/fc_rs(t-1) instead of in a
  serial 45us tail; its ReduceScatter fired ~1.5 tiles earlier, which covers
  cross-core skew so the DVE FIFO never head-of-line blocks on the
  collective.  fc parts split 4+4 between A(0,t) and A(1,t) as PE fillers
  (emitted BEFORE the scores, which must stay first -- they are the
  always-ready PE work).
- single 2MB input DMAs (sub-1MiB transfers are descriptor-dominated), fc
  descale alternates ACT/DVE, denominator-row copy on ACT.
- known dead ends (measured): stride-0 DRAM-bounce broadcast for the softmax
  denominator races (hazard tracker misses the ordering -> NaN); row-group
  score-MM pairs never co-execute (0/255 overlaps); ctx-before-scores order
  head-of-line blocks the PE (+60us); uint8 DoublePixel is rejected by bass.
"""

import numpy as np
import ml_dtypes
from contextlib import ExitStack

B = 2
N = 2048
D = 1024
H = 16
DK = 64
HL = H // 4          # heads per core
CSL = HL * DK        # 256 per-core fc contraction
ROWS = N // 4        # 512 output rows per core
LN_EPS = 1e-6
N_CORES = 8
WFC_SCALE = 64.0     # host pre-scale on w_fc (fp8 subnormal dodge)
WQK_SCALE = 64.0     # host pre-scale on w_qs/w_ks -> scores come out x4096
WV_SCALE = 32.0      # host pre-scale on w_vs -> vh/ctx come out x32

_CACHE = {}


def _build():
    import concourse.bass as bass
    import concourse.tile as tile
    import concourse.mybir as mybir
    from concourse import bacc

    bf16 = mybir.dt.bfloat16
    f32 = mybir.dt.float32
    fp8 = mybir.dt.float8e4
    AF = mybir.ActivationFunctionType
    Alu = mybir.AluOpType
    DR = mybir.MatmulPerfMode.DoubleRow

    nc = bacc.Bacc("TRN2", target_bir_lowering=False, debug=False,
                   num_devices=N_CORES)

    KCl = D // 128
    # weights/residual arrive pre-arranged from the host in SBUF layout so
    # their DMAs are contiguous 2-4KB-per-partition line-rate transfers
    qT = nc.dram_tensor("qT", [D, N], fp8, kind="ExternalInput").ap()
    kT = nc.dram_tensor("kT", [D, N], fp8, kind="ExternalInput").ap()
    vT = nc.dram_tensor("vT", [D, N], fp8, kind="ExternalInput").ap()
    wq = nc.dram_tensor("wq", [128, KCl, CSL], fp8, kind="ExternalInput").ap()
    wk = nc.dram_tensor("wk", [128, KCl, CSL], fp8, kind="ExternalInput").ap()
    wv = nc.dram_tensor("wv", [128, KCl, CSL], fp8, kind="ExternalInput").ap()
    wfc8 = nc.dram_tensor("wfc8", [128, CSL // 128, D], fp8, kind="ExternalInput").ap()
    qres = nc.dram_tensor("qres", [128, N // 512, D], f32, kind="ExternalInput").ap()
    gamma = nc.dram_tensor("gamma", [D], f32, kind="ExternalInput").ap()
    beta = nc.dram_tensor("beta", [D], f32, kind="ExternalInput").ap()
    y = nc.dram_tensor("y", [ROWS, D], f32, kind="ExternalOutput").ap()

    KC = D // 128     # 8 contraction chunks for projections
    KCP = KC // 2     # 4 DoubleRow chunk-pairs for projections
    ST = N // 512     # 4 seq tiles of 512 queries
    SC = N // 128     # 16 seq chunks of 128 keys
    G = 2             # key chunks per exp batch / DR pair
    NG = SC // G      # 8 groups per attention tile
    # raw scores in PSUM are x(WQK_SCALE^2); fold into the exp scale exactly
    EXP_SCALE = 1.0 / (float(np.sqrt(DK)) * WQK_SCALE * WQK_SCALE)
    # Schraudolph exp straight into e4m3 bits: u8 = round(A8*x + B8) as int8,
    # bitcast to fp8.  One DVE tensor_scalar per slice; numpy sim shows the
    # softmax cancels the approx error (rel 1.1e-3, same as exact exp).
    EXP8_A = 8.0 * float(np.log2(np.e)) * EXP_SCALE
    EXP8_B = 8.0 * (7.0 - 0.45)

    def exp_on_dve(g, s):
        # 4 of 16 slices per attention tile go to the DVE, spread out so
        # neither engine gets a long exclusive run
        return g % 2 == 1 and s == (g // 2) % 2

    with tile.TileContext(nc) as tc:
        with ExitStack() as ctx:
            persist = ctx.enter_context(tc.tile_pool(name="persist", bufs=1))
            work = ctx.enter_context(tc.tile_pool(name="work", bufs=2))
            epool = ctx.enter_context(tc.tile_pool(name="epool", bufs=4))
            pat = ctx.enter_context(tc.tile_pool(name="pat", bufs=1, space="PSUM"))
            dram = ctx.enter_context(tc.tile_pool(name="dram", bufs=2, space="DRAM"))
            late_ctx = ExitStack()
            late = late_ctx.enter_context(tc.tile_pool(name="late", bufs=1))
            qkv_ctx = ExitStack()
            qkv = qkv_ctx.enter_context(tc.tile_pool(name="qkv", bufs=1))

            # PSUM: "s" [128,2,512] x3 = 6 banks (scores; proj/fc borrow these
            # slots), "c" [65,512] x2 = 2 banks (ctx accumulators).
            def ps_s():
                return pat.tile([128, G, 512], f32, tag="s", name="ps_s", bufs=3)

            def ps_c():
                # full 128 partitions: ctx uses [0:DK+1], the HAM keepalive
                # matmuls write the otherwise-unused [96:128] strip
                return pat.tile([128, 512], f32, tag="c", name="ps_c", bufs=2)

            def ps_f(n=512):
                return pat.tile([128, n], f32, tag="s", name="ps_f", bufs=3)

            # ---- input loads: weights first, then per-slab slices ------------
            wq_sb = qkv.tile([128, KC, CSL], fp8, tag="wq", name="wq")
            wk_sb = qkv.tile([128, KC, CSL], fp8, tag="wk", name="wk")
            wv_sb = qkv.tile([128, KC, CSL], fp8, tag="wv", name="wv")
            nc.sync.dma_start(out=wq_sb, in_=wq)
            nc.sync.dma_start(out=wk_sb, in_=wk)

            qT_sb = qkv.tile([128, KC, N], fp8, tag="qT", name="qT")
            kT_sb = qkv.tile([128, KC, N], fp8, tag="kT", name="kT")
            vT_sb = qkv.tile([128, KC, N], fp8, tag="vT", name="vT")
            # halves interleaved: q/k half 0 first (first proj + attention
            # groups 0-3), then v half 0 (vhp for those groups), then the
            # second halves
            # one 2MB DMA per tensor (transfers under ~1MiB are descriptor-
            # dominated): out[p, kc, n] = in[kc*128 + p, n]
            def xT_ap(x):
                return bass.AP(tensor=x.tensor, offset=x.offset,
                               ap=[[N, 128], [128 * N, KC], [1, N]])

            nc.sync.dma_start(out=qT_sb, in_=xT_ap(qT))
            nc.sync.dma_start(out=kT_sb, in_=xT_ap(kT))
            nc.sync.dma_start(out=wv_sb, in_=wv)
            nc.sync.dma_start(out=vT_sb, in_=xT_ap(vT))

            # late inputs on the scalar HWDGE ring so they don't delay the above
            wfc_sb = late.tile([128, CSL // 128, D], fp8, tag="wfc", name="wfc")
            nc.scalar.dma_start(out=wfc_sb, in_=wfc8)
            qres_sb = late.tile([128, ST, D], f32, tag="qres", name="qres")
            nc.scalar.dma_start(out=qres_sb, in_=qres)
            gamma_sb = late.tile([128, D], f32, tag="gamma", name="gamma")
            nc.scalar.dma_start(out=gamma_sb,
                                in_=bass.AP(tensor=gamma.tensor, offset=gamma.offset,
                                            ap=[[0, 128]] + gamma.ap))
            beta_sb = late.tile([128, D], f32, tag="beta", name="beta")
            nc.scalar.dma_start(out=beta_sb,
                                in_=bass.AP(tensor=beta.tensor, offset=beta.offset,
                                            ap=[[0, 128]] + beta.ap))
            xacc = qres_sb  # relu+residual accumulates in place over the residual

            # HAM warm-up: dependency-free matmuls on memset data span the
            # input-DMA wait so the PE clock is at 2.4GHz (not the cold
            # 1.2GHz) when the first projection matmuls arrive, and the
            # 4096-cycle activity window never lapses during the DMA-paced
            # startup phase.
            warm = qkv.tile([128, 512], bf16, tag="warm", name="warm")
            nc.vector.memset(warm, 0.0)
            for _ in range(48):
                nc.tensor.matmul(ps_f()[:, 0:256], warm[:, 0:128],
                                 warm[:, 0:256], start=True, stop=True)

            # ---- projections -------------------------------------------------
            # qhT/khT: [dk, seq] per head, heads 2p / 2p+1 stacked on partition
            # halves.  k copies go through the scalar engine pre-attention so
            # DVE and ACT split the psum-evacuation work.
            qhT = [persist.tile([128, N], bf16, tag=f"qhT{p}", name=f"qhT{p}") for p in range(2)]
            khT = [persist.tile([128, N], bf16, tag=f"khT{p}", name=f"khT{p}") for p in range(2)]

            def qk_proj_pair(p, st, use_act=False):
                # q and k chains interleaved on two psum banks: consecutive
                # MMs never hit the same bank (WAW chain penalty) and LDW
                # overlaps the previous matmul.  fp8 DoubleRow: 256 of the
                # 1024 contraction dims per MM (chunk-pairs 2kcp/2kcp+1).
                sl = slice(st * 512, (st + 1) * 512)
                psq = ps_f()
                psk = ps_f()
                for kcp in range(KCP):
                    cp = slice(2 * kcp, 2 * kcp + 2)
                    nc.tensor.matmul(
                        psq, wq_sb[:, cp, p * 128:(p + 1) * 128],
                        qT_sb[:, cp, sl],
                        start=(kcp == 0), stop=(kcp == KCP - 1),
                        perf_mode=DR)
                    nc.tensor.matmul(
                        psk, wk_sb[:, cp, p * 128:(p + 1) * 128],
                        kT_sb[:, cp, sl],
                        start=(kcp == 0), stop=(kcp == KCP - 1),
                        perf_mode=DR)
                nc.vector.tensor_copy(out=qhT[p][:, sl], in_=psq)
                if use_act:
                    nc.scalar.copy(out=khT[p][:, sl], in_=psk)
                else:
                    nc.vector.tensor_copy(out=khT[p][:, sl], in_=psk)

            # vh pairs for DoubleRow ctx: [128 keys, 2 chunks, HL heads, 80]
            # (65 used: 64 dims + ones column; 80 keeps the DR pair stride
            # 16B-aligned).
            vhp = [persist.tile([128, G, HL, 80], fp8, tag=f"vhp{g}", name=f"vhp{g}")
                   for g in range(NG)]
            for g in range(NG):
                nc.vector.memset(vhp[g][:, :, :, DK:DK + 1], 1.0)

            def v_proj_pair(g):
                # chunks 2g / 2g+1 interleaved on two psum banks (fp8 DR)
                sc0 = 2 * g
                psa = ps_f(CSL)
                psb = ps_f(CSL)
                for kcp in range(KCP):
                    cp = slice(2 * kcp, 2 * kcp + 2)
                    nc.tensor.matmul(
                        psa, vT_sb[:, cp, sc0 * 128:(sc0 + 1) * 128],
                        wv_sb[:, cp, :],
                        start=(kcp == 0), stop=(kcp == KCP - 1),
                        perf_mode=DR)
                    nc.tensor.matmul(
                        psb, vT_sb[:, cp, (sc0 + 1) * 128:(sc0 + 2) * 128],
                        wv_sb[:, cp, :],
                        start=(kcp == 0), stop=(kcp == KCP - 1),
                        perf_mode=DR)
                nc.vector.tensor_copy(
                    out=vhp[g][:, 0, :, 0:DK],
                    in_=psa.rearrange("p (h d) -> p h d", h=HL))
                nc.vector.tensor_copy(
                    out=vhp[g][:, 1, :, 0:DK],
                    in_=psb.rearrange("p (h d) -> p h d", h=HL))

            # ctxn: normalized context, fp8, [128 c, 2 cc-chunks, seq] --
            # cc chunk index == p (heads 2p, 2p+1 on partition halves).
            ctxn = persist.tile([128, 2, N], fp8, tag="ctxn", name="ctxn")
            ones_sb = persist.tile([1, DK], bf16, tag="ones", name="ones")
            nc.vector.memset(ones_sb, 1.0)

            def attention(p, t, extra=None):
                pc = [ps_c() for _ in range(2)]
                pse_prev = [None, None]

                def ctx_mm(g, s):
                    nc.tensor.matmul(
                        pc[s][0:DK + 1, :],
                        vhp[g][:, :, 2 * p + s, 0:DK + 1],
                        pse_prev[s],
                        start=(g == 0), stop=(g == NG - 1),
                        perf_mode=DR)

                for g in range(NG):
                    # fillers first: they are dependency-free, so they run
                    # while the scores MMs behind them wait on psum drain --
                    # keeping the PE busy (HAM stays at full clock)
                    if extra is not None:
                        extra(g)
                    if g >= 1:
                        # HAM keepalive: a tiny dependency-free matmul into
                        # the unused partitions 96-127 of the ctx accumulator
                        # bank.  It absorbs the PE stall at the psum-slot
                        # rotation so the activity monitor never re-throttles
                        # the clock to 1.2GHz mid-kernel.
                        nc.tensor.matmul(
                            pc[g % 2][96:128, 0:256], warm[:, 0:32],
                            warm[:, 0:256], start=True, stop=True,
                            tile_position=(0, 96), skip_group_check=True)
                    ppss = [ps_s(), ps_s()]
                    # scores: alternate row groups every MM so LDW overlaps
                    for j in range(G):
                        kc = g * G + j
                        for s in range(2):
                            lo = 64 * s
                            nc.tensor.matmul(
                                ppss[s][:, j, :],
                                khT[p][lo:lo + 64, kc * 128:(kc + 1) * 128],
                                qhT[p][lo:lo + 64, t * 512:(t + 1) * 512],
                                start=True, stop=True,
                                tile_position=(lo, 0))
                    # ctx for the previous group (exp ran during these scores)
                    if g > 0:
                        for s in range(2):
                            ctx_mm(g - 1, s)
                    for s in range(2):
                        if exp_on_dve(g, s):
                            u8 = epool.tile([128, G, 512], mybir.dt.int8,
                                            tag="e8", name="e8")
                            nc.vector.tensor_scalar(
                                out=u8, in0=ppss[s], scalar1=EXP8_A,
                                scalar2=EXP8_B, op0=Alu.mult, op1=Alu.add)
                            pse_prev[s] = u8.bitcast(fp8)
                        else:
                            pse = epool.tile([128, G, 512], fp8, tag="e", name="e")
                            nc.scalar.activation(out=pse, in_=ppss[s], func=AF.Exp,
                                                 scale=EXP_SCALE)
                            pse_prev[s] = pse
                for s in range(2):
                    ctx_mm(NG - 1, s)

                # normalize: broadcast the ones-row (softmax denominator)
                # across 64 partitions with a K=1 ones-matmul on the PE, take
                # the fast approx reciprocal (1.2 cpe vs 6), multiply into the
                # fp8 ctxn tile.  (A stride-0 DRAM-bounce broadcast would free
                # the PE matmul, but the hazard tracker misses the ordering on
                # the stride-0 read -> races -> NaN.  Keep the PE path.)
                for s in range(2):
                    d1 = work.tile([1, 512], bf16, tag="rb", name="rb")
                    nc.scalar.copy(out=d1, in_=pc[s][DK:DK + 1, :])
                    ps_rb = ps_f()
                    nc.tensor.matmul(ps_rb[0:DK, :], ones_sb, d1,
                                     start=True, stop=True)
                    rb = work.tile([DK, 512], f32, tag="rb64", name="rb64")
                    nc.vector.reciprocal_approx_fast(out=rb, in_=ps_rb[0:DK, :])
                    nc.vector.tensor_mul(
                        out=ctxn[64 * s:64 * (s + 1), p, t * 512:(t + 1) * 512],
                        in0=pc[s][0:DK, :], in1=rb)

            # fc (fp8 DoubleRow, both 128-chunks contracted per MM) + chunked
            # ReduceScatter per slab; each core ends with rows
            # [t*512 + rank*128, +128) of its batch.
            def fc_mm(t, qq, nh):
                qc = t * 4 + qq
                ps = ps_f()
                nc.tensor.matmul(
                    ps,
                    ctxn[:, :, qc * 128:(qc + 1) * 128],
                    wfc_sb[:, :, nh * 512:(nh + 1) * 512],
                    start=True, stop=True, perf_mode=DR)
                fcs = work.tile([128, 512], fp8, tag="fcs", name="fcs", bufs=8)
                # descale alternates engines to balance ACT vs DVE load
                if (qq + nh) % 2 == 0:
                    nc.scalar.activation(out=fcs, in_=ps, func=AF.Copy,
                                         scale=1.0 / (WFC_SCALE * WV_SCALE))
                else:
                    nc.vector.tensor_scalar(out=fcs, in0=ps,
                                            scalar1=1.0 / (WFC_SCALE * WV_SCALE),
                                            scalar2=None, op0=Alu.mult)
                return fcs

            rs_outs = []

            def fc_rs(t, fcs_parts):
                rs_in = dram.tile([512, D], fp8, tag="rs_in", name="rs_in",
                                  bufs=4)
                rs_out = dram.tile([128, D], fp8, tag="rs_out", name="rs_out",
                                   bufs=4)
                for (qq, nh), fcs in fcs_parts.items():
                    nc.sync.dma_start(
                        out=rs_in[qq * 128:(qq + 1) * 128, nh * 512:(nh + 1) * 512],
                        in_=fcs)
                nc.gpsimd.collective_compute(
                    "ReduceScatter",
                    mybir.AluOpType.add,
                    replica_groups=[[0, 1, 2, 3], [4, 5, 6, 7]],
                    ins=[rs_in.opt()],
                    outs=[rs_out.opt()])
                rs_outs.append(rs_out)

            # ---- relu/residual + layernorm for one 128-row output slab ------
            # rstd on the DVE (bit-trick seed + 2 Newton iterations) keeps the
            # scalar engine's ACT table on the exp set for the whole kernel.
            i32 = mybir.dt.int32
            MAGIC = float(0x5F3759DF)

            def ln_slab(t, gate):
                rs_sb = work.tile([128, D], fp8, tag="rs_sb", name="rs_sb",
                                  bufs=2)
                # corner-write gate: a data dep on attention progress keeps
                # this collective-waiting load (and the DVE ops behind it)
                # from hoisting into earlier engine-FIFO slots.
                nc.gpsimd.tensor_copy(out=rs_sb[0:1, 0:1], in_=gate)
                nc.gpsimd.dma_start(out=rs_sb, in_=rs_outs[t])
                nc.vector.scalar_tensor_tensor(
                    out=xacc[:, t, :], in0=rs_sb, scalar=0.0,
                    in1=qres_sb[:, t, :], op0=Alu.max, op1=Alu.add)
                x = xacc[:, t, :]
                stats = work.tile([128, 2, 6], f32, tag="stats", name="stats")
                nc.vector.bn_stats(out=stats[:, 0, :], in_=x[:, 0:512])
                nc.vector.bn_stats(out=stats[:, 1, :], in_=x[:, 512:1024])
                mv = work.tile([128, 2], f32, tag="mv", name="mv")
                nc.vector.bn_aggr(out=mv, in_=stats)
                v = work.tile([128, 1], f32, tag="veps", name="veps")
                nc.vector.tensor_scalar(out=v, in0=mv[:, 1:2], scalar1=LN_EPS,
                                        scalar2=None, op0=Alu.add)
                # rsqrt: y0 = bitcast(0x5F3759DF - bits(v)/2) via f32 arithmetic
                # on the bit pattern (exact enough for a seed), then 2x Newton.
                si = work.tile([128, 1], i32, tag="rss", name="rss")
                nc.vector.tensor_scalar(out=si, in0=v.bitcast(i32), scalar1=-0.5,
                                        scalar2=MAGIC, op0=Alu.mult, op1=Alu.add)
                yk = si.bitcast(f32)
                for _ in range(2):
                    a = work.tile([128, 1], f32, tag="rsa", name="rsa")
                    nc.vector.tensor_mul(out=a, in0=yk, in1=yk)
                    nc.vector.tensor_mul(out=a, in0=a, in1=v)
                    nc.vector.tensor_scalar(out=a, in0=a, scalar1=-0.5,
                                            scalar2=1.5, op0=Alu.mult, op1=Alu.add)
                    yn = work.tile([128, 1], f32, tag="rsy", name="rsy")
                    nc.vector.tensor_mul(out=yn, in0=yk, in1=a)
                    yk = yn
                xo = work.tile([128, D], f32, tag="xo", name="xo")
                nc.vector.tensor_scalar(out=xo, in0=x,
                                        scalar1=mv[:, 0:1], scalar2=yk,
                                        op0=Alu.subtract, op1=Alu.mult)
                nc.vector.tensor_mul(out=xo, in0=xo, in1=gamma_sb)
                nc.vector.tensor_add(out=xo, in0=xo, in1=beta_sb)
                nc.sync.dma_start(out=y[t * 128:(t + 1) * 128, :], in_=xo)

            # ---- schedule ----------------------------------------------------
            for st in range(ST):
                qk_proj_pair(0, st, use_act=True)
            v_proj_pair(0)
            v_proj_pair(1)

            # A(0,0) group g consumes vhp[g]; emit vhp[g+2] at group g, then
            # start qk_proj(1) in the last two group slots.
            def extra00(g):
                if g + 2 < NG:
                    v_proj_pair(g + 2)
                elif g == 6:
                    qk_proj_pair(1, 0)
                elif g == 7:
                    qk_proj_pair(1, 1)
            attention(0, 0, extra=extra00)

            def extra10(g):
                if g == 2:
                    qk_proj_pair(1, 2)
                elif g == 5:
                    qk_proj_pair(1, 3)
            attention(1, 0, extra=extra10)

            # fc for slab t-1: 4 parts as PE fillers in each of A(0,t) and
            # A(1,t) so both attention tiles keep the PE dense; ReduceScatter
            # fires after A(1,t).  LayerNorm for slab t-2 interleaves between
            # A(0,t) and A(1,t): its collective fired ~1.5 attention tiles
            # earlier, which covers cross-core skew, so the DVE ops behind it
            # don't head-of-line block.
            fcs_pending = {}

            def extra_fc(t_prev, half):
                def fill(g):
                    if g % 2 == 1:
                        part = 4 * half + g // 2
                        qq, nh = divmod(part, 2)
                        fcs_pending[(qq, nh)] = fc_mm(t_prev, qq, nh)
                return fill

            for t in range(1, ST):
                attention(0, t, extra=extra_fc(t - 1, 0))
                attention(1, t, extra=extra_fc(t - 1, 1))
                fc_rs(t - 1, fcs_pending)
                fcs_pending = {}
                if t >= 2:
                    ln_slab(t - 2, ctxn[0:1, 1, (t + 1) * 512 - 1:(t + 1) * 512])
            for part in range(8):
                qq, nh = divmod(part, 2)
                fcs_pending[(qq, nh)] = fc_mm(ST - 1, qq, nh)
            fc_rs(ST - 1, fcs_pending)
            qkv_ctx.close()

            # ---- tail: last two layernorm slabs ------------------------------
            last_gate = ctxn[0:1, 1, N - 1:N]
            ln_slab(2, last_gate)
            ln_slab(3, last_gate)
            late_ctx.close()

    nc.compile()
    return nc


def kernel(q, k, v, w_qs, w_ks, w_vs, w_fc, ln_gamma, ln_beta):
    from concourse import bass_utils

    if "nc" not in _CACHE:
        _CACHE["nc"] = _build()
    nc = _CACHE["nc"]

    bf = ml_dtypes.bfloat16
    f8 = ml_dtypes.float8_e4m3
    q = np.asarray(q, np.float32)
    k = np.asarray(k, np.float32)
    v = np.asarray(v, np.float32)
    w_fc = np.asarray(w_fc, np.float32)

    def to_f8(a):
        return np.clip(a, -448.0, 448.0).astype(f8)

    def warr(w, cs, scale):
        # [D, CSL] -> SBUF layout [128, KC, CSL], pre-scaled fp8
        a = np.asarray(w, np.float32)[:, cs].reshape(D // 128, 128, CSL) * scale
        return to_f8(np.ascontiguousarray(a.transpose(1, 0, 2)))

    in_maps = []
    for i in range(N_CORES):
        bi, hg = i // 4, i % 4
        cs = slice(hg * CSL, (hg + 1) * CSL)
        row_idx = np.concatenate(
            [np.arange(t * 512 + hg * 128, t * 512 + (hg + 1) * 128) for t in range(4)])
        wfc_a = (w_fc[cs, :] * WFC_SCALE).reshape(CSL // 128, 128, D)
        qres_a = q[bi][row_idx].reshape(4, 128, D)
        in_maps.append({
            "qT": to_f8(np.ascontiguousarray(q[bi].T)),
            "kT": to_f8(np.ascontiguousarray(k[bi].T)),
            "vT": to_f8(np.ascontiguousarray(v[bi].T)),
            "wq": warr(w_qs, cs, WQK_SCALE),
            "wk": warr(w_ks, cs, WQK_SCALE),
            "wv": warr(w_vs, cs, WV_SCALE),
            "wfc8": to_f8(np.ascontiguousarray(wfc_a.transpose(1, 0, 2))),
            "qres": np.ascontiguousarray(qres_a.transpose(1, 0, 2)).astype(np.float32),
            "gamma": np.ascontiguousarray(np.asarray(ln_gamma, np.float32)),
            "beta": np.ascontiguousarray(np.asarray(ln_beta, np.float32)),
        })

    run_kwargs = dict(_CACHE.get("run_kwargs", {}))
    res = bass_utils.run_bass_kernel_spmd(nc, in_maps, core_ids=list(range(N_CORES)),
                                          **run_kwargs)
    _CACHE["last_res"] = res
    out = np.empty((B, N, D), np.float32)
    for i in range(N_CORES):
        bi, hg = i // 4, i % 4
        yi = res.results[i]["y"]
        for t in range(4):
            out[bi, t * 512 + hg * 128:t * 512 + (hg + 1) * 128, :] = \
                yi[t * 128:(t + 1) * 128, :]
    return out

